# revision 26
# baseline (speedup 1.0000x reference)
"""Trainium2 Bass kernel for nn_MultiHeadAttention (N=8, S=1024, E=1024, H=16).

Strategy: pure data-parallel over the batch dim N=8 -> one batch element per
NeuronCore, no collectives. Per core the whole MHA runs out of SBUF:

  v   = xv @ Wv_aug.T + bv_aug   (S-major [S, H*(D+1)] with a ones column
                                  appended per head -> o-matmul also yields
                                  the softmax denominator for free)
  q.T = Wq @ xq.T + bq      (E-major "transposed" layout [E, S])
  k.T = Wk @ xk.T + bk
  per head h (software-pipelined at s_k-tile granularity):
    scoresT[s_k, s_q] tile = k_h.T-slice x q_h.T   (PSUM)
    attnT = exp(scoresT/sqrt(E))  on ScalarE (the only exp engine; paces
                                   the attention phase at ~1us/tile)
    o_unnorm.T[d, s_q] (+ denom row) += v_aug_h^T @ attnT  (PSUM accum)
  normalization is deferred/batched: denominator rows collect in SBUF, a
  fast approx reciprocal runs per batch of heads, the reciprocal rows are
  broadcast across partitions with a tiny K=2 fp32 matmul (selector
  constant), and oT is scaled in place -- all off the PE critical path.
  out = o @ Wo.T + bo       (natural [S, E] layout, DMA to DRAM)

All big matmul operands are bf16 (fp32 accumulation in PSUM); inputs are
pre-transposed and pre-cast on the host (layout/precision prep only).
"""

import math
from contextlib import ExitStack

import numpy as np

P = 128  # SBUF partitions
FDMAX = 512  # matmul moving-operand free-dim tile

_NC_CACHE = {}


def _emit(ctx, tc, io, S, E, H):
    from concourse import mybir

    nc = tc.nc
    D = E // H
    DA = D + 1
    HA = H * DA
    NTE = E // P  # partition tiles over e/f dims
    NTS = S // P  # partition tiles over s dim
    FD = min(FDMAX, S)
    NQ = S // FD  # free tiles over s
    NE = E // FD  # free tiles over e
    HPT = P // D  # heads per e-tile
    f32 = mybir.dt.float32
    bf16 = mybir.dt.bfloat16

    singles = ctx.enter_context(tc.tile_pool(name="singles", bufs=1))
    xpool = ctx.enter_context(tc.tile_pool(name="xpool", bufs=2))
    wpool = ctx.enter_context(tc.tile_pool(name="wpool", bufs=2))
    atp = ctx.enter_context(tc.tile_pool(name="atp", bufs=4))
    outp = ctx.enter_context(tc.tile_pool(name="outp", bufs=2))
    mini = ctx.enter_context(tc.tile_pool(name="mini", bufs=2))
    psA = ctx.enter_context(tc.tile_pool(name="psA", bufs=4, space="PSUM"))

    # persistent bf16 activations; layout [row % P, tile_idx * width + col]
    qT = singles.tile([P, NTE * S], bf16)  # q.T [e, s]
    kT = singles.tile([P, NTE * S], bf16)  # k.T [e, s]
    vA = singles.tile([P, NTS * HA], bf16)  # v_aug [s, HA]
    oT = singles.tile([P, NTE * S], bf16)  # o.T [e, s]

    # ---- input loads, in consumption order (DMA queues drain in order) ----
    xv_bf = xpool.tile([P, NTE * S], bf16, tag="x")
    wv_bf = wpool.tile([P, NTE * max(E, HA)], bf16, tag="w")
    for t in range(NTE):
        nc.sync.dma_start(
            out=wv_bf[:, t * HA : (t + 1) * HA], in_=io["wvTa"][t * P : (t + 1) * P, :]
        )
        nc.gpsimd.dma_start(
            out=xv_bf[:, t * S : (t + 1) * S], in_=io["xvT"][t * P : (t + 1) * P, :]
        )
    xq_bf = xpool.tile([P, NTE * S], bf16, tag="x")
    wq_bf = wpool.tile([P, NTE * max(E, HA)], bf16, tag="w")
    for t in range(NTE):
        nc.sync.dma_start(
            out=wq_bf[:, t * E : (t + 1) * E], in_=io["wqT"][t * P : (t + 1) * P, :]
        )
        nc.gpsimd.dma_start(
            out=xq_bf[:, t * S : (t + 1) * S], in_=io["xqT"][t * P : (t + 1) * P, :]
        )
    xk_bf = xpool.tile([P, NTE * S], bf16, tag="x")
    wk_bf = wpool.tile([P, NTE * max(E, HA)], bf16, tag="w")
    for t in range(NTE):
        nc.sync.dma_start(
            out=wk_bf[:, t * E : (t + 1) * E], in_=io["wkT"][t * P : (t + 1) * P, :]
        )
        nc.gpsimd.dma_start(
            out=xk_bf[:, t * S : (t + 1) * S], in_=io["xkT"][t * P : (t + 1) * P, :]
        )
    wo_bf = wpool.tile([P, NTE * max(E, HA)], bf16, tag="w")
    for t in range(NTE):
        eng = nc.sync if t % 2 == 0 else nc.gpsimd
        eng.dma_start(
            out=wo_bf[:, t * E : (t + 1) * E], in_=io["woT"][t * P : (t + 1) * P, :]
        )

    # biases + selector constant (scalar queue; small)
    bq_sb = singles.tile([P, NTE], f32)
    bk_sb = singles.tile([P, NTE], f32)
    bq2 = io["bq"].rearrange("(a b) -> a b", b=1)
    bk2 = io["bk"].rearrange("(a b) -> a b", b=1)
    for et in range(NTE):
        nc.scalar.dma_start(
            out=bq_sb[:, et : et + 1], in_=bq2[et * P : (et + 1) * P, :]
        )
        nc.scalar.dma_start(
            out=bk_sb[:, et : et + 1], in_=bk2[et * P : (et + 1) * P, :]
        )
    bva_sb = singles.tile([P, HA], f32)
    nc.scalar.dma_start(
        out=bva_sb, in_=io["bva"].rearrange("(a b) -> a b", a=1).to_broadcast((P, HA))
    )
    bo_sb = singles.tile([P, E], f32)
    nc.scalar.dma_start(
        out=bo_sb, in_=io["bo"].rearrange("(a b) -> a b", a=1).to_broadcast((P, E))
    )
    sel_sb = singles.tile([HPT, P], f32)
    nc.scalar.dma_start(out=sel_sb, in_=io["sel"])

    # ---- v projection: v_aug[s, c] = sum_f xv[f, s] * wv_aug[f, c] + bva ----
    main_w = (HA // FD) * FD
    tail_w = HA - main_w
    for st_i in range(NTS):
        ps_main = psA.tile([P, max(S, main_w)], f32, tag="psA")
        ps_tail = (
            psA.tile([P, max(S, main_w)], f32, tag="psA", name=f"ps_tail_{st_i}")
            if tail_w
            else None
        )
        for kt in range(NTE):
            lhsT = xv_bf[:, kt * S + st_i * P : kt * S + st_i * P + P]
            for j in range(main_w // FD):
                nc.tensor.matmul(
                    ps_main[:, j * FD : (j + 1) * FD],
                    lhsT,
                    wv_bf[:, kt * HA + j * FD : kt * HA + (j + 1) * FD],
                    start=(kt == 0),
                    stop=(kt == NTE - 1),
                )
            if ps_tail is not None:
                nc.tensor.matmul(
                    ps_tail[:, :tail_w],
                    lhsT,
                    wv_bf[:, kt * HA + main_w : kt * HA + HA],
                    start=(kt == 0),
                    stop=(kt == NTE - 1),
                )
        nc.vector.tensor_add(
            out=vA[:, st_i * HA : st_i * HA + main_w],
            in0=ps_main[:, :main_w],
            in1=bva_sb[:, :main_w],
        )
        if ps_tail is not None:
            nc.vector.tensor_add(
                out=vA[:, st_i * HA + main_w : (st_i + 1) * HA],
                in0=ps_tail[:, :tail_w],
                in1=bva_sb[:, main_w:HA],
            )

    # ---- q/k projections: dst[e, s] = sum_f w[f, e] x[f, s] + b[e] ----
    def project_qk(dst, w_bf, x_bf, bias_sb):
        for et in range(NTE):
            ps = psA.tile([P, max(S, main_w)], f32, tag="psA")
            for kt in range(NTE):
                lhsT = w_bf[:, kt * E + et * P : kt * E + (et + 1) * P]
                for j in range(NQ):
                    nc.tensor.matmul(
                        ps[:, j * FD : (j + 1) * FD],
                        lhsT,
                        x_bf[:, kt * S + j * FD : kt * S + (j + 1) * FD],
                        start=(kt == 0),
                        stop=(kt == NTE - 1),
                    )
            nc.vector.tensor_scalar_add(
                out=dst[:, et * S : (et + 1) * S],
                in0=ps[:, :S],
                scalar1=bias_sb[:, et : et + 1],
            )

    project_qk(qT, wq_bf, xq_bf, bq_sb)
    project_qk(kT, wk_bf, xk_bf, bk_sb)

    # ---- attention, kt-granular software pipeline per head ----
    inv_scale = 1.0 / math.sqrt(E)
    nbatch = 4 if H % 4 == 0 else 1
    hb = H // nbatch
    den_batches = [
        singles.tile([hb, S], f32, name=f"den_batch{b}") for b in range(nbatch)
    ]

    def normalize_et(et):
        # rb[p, s] = 1/den[head(p), s] replicated via K=HPT fp32 matmul
        den2 = mini.tile([HPT, S], f32, tag="den2", name=f"den2_{et}")
        for i in range(HPT):
            hh = et * HPT + i
            nc.gpsimd.dma_start(
                out=den2[i : i + 1, :],
                in_=den_batches[hh // hb][hh % hb : hh % hb + 1, :],
            )
        rb = psA.tile([P, max(S, main_w)], f32, tag="psA", name=f"rb_{et}")
        for j in range(NQ):
            nc.tensor.matmul(
                rb[:, j * FD : (j + 1) * FD],
                sel_sb,
                den2[:, j * FD : (j + 1) * FD],
                start=True,
                stop=True,
            )
        nc.vector.tensor_mul(
            out=oT[:, et * S : (et + 1) * S],
            in0=oT[:, et * S : (et + 1) * S],
            in1=rb[:, :S],
        )

    for h in range(H):
        eh = (h * D) // P
        ph = (h * D) % P
        pso = psA.tile([P, max(S, main_w)], f32, tag="psA", name=f"pso_{h}")
        ats = []
        # scores/exp for kt, with o-matmuls trailing one kt behind
        for kt in range(NTS):
            ps = psA.tile([P, max(S, main_w)], f32, tag="psA", name=f"sc_{h}_{kt}")
            lhsT = kT[ph : ph + D, eh * S + kt * P : eh * S + (kt + 1) * P]
            for j in range(NQ):
                nc.tensor.matmul(
                    ps[:, j * FD : (j + 1) * FD],
                    lhsT,
                    qT[ph : ph + D, eh * S + j * FD : eh * S + (j + 1) * FD],
                    start=True,
                    stop=True,
                )
            at = atp.tile([P, S], bf16, tag="at", name=f"at_{h}_{kt}")
            nc.scalar.activation(
                out=at,
                in_=ps[:, :S],
                func=mybir.ActivationFunctionType.Exp,
                scale=inv_scale,
            )
            ats.append(at)

            def o_mms(okt):
                lhsTo = vA[:, okt * HA + h * DA : okt * HA + (h + 1) * DA]
                for j in range(NQ):
                    nc.tensor.matmul(
                        pso[:DA, j * FD : (j + 1) * FD],
                        lhsTo,
                        ats[okt][:, j * FD : (j + 1) * FD],
                        start=(okt == 0),
                        stop=(okt == NTS - 1),
                    )

            if kt >= 1:
                o_mms(kt - 1)
        o_mms(NTS - 1)
        # evacuate unnormalized o (bf16) + denominator row
        nc.vector.tensor_copy(
            out=oT[ph : ph + D, eh * S : (eh + 1) * S], in_=pso[:D, :S]
        )
        den_tmp = mini.tile([1, S], f32, tag="den_tmp")
        nc.vector.tensor_copy(out=den_tmp, in_=pso[D:DA, :S])
        nc.gpsimd.dma_start(
            out=den_batches[h // hb][h % hb : h % hb + 1, :], in_=den_tmp
        )
        if h % hb == hb - 1:
            b = h // hb
            nc.vector.reciprocal_approx_fast(
                out=den_batches[b], in_=den_batches[b]
            )
            # normalize e-tiles fully covered so far (overlaps later heads)
            et_lo = (b * hb) // HPT
            et_hi = (h + 1) // HPT
            for et in range(et_lo, et_hi):
                normalize_et(et)

    # ---- output projection: out[s, e] = sum_f oT[f, s] woT[f, e] + bo ----
    for st_i in range(NTS):
        osb = outp.tile([P, E], f32, tag="out", name=f"osb_{st_i}")
        ps = psA.tile([P, max(S, main_w)], f32, tag="psA", name=f"po_{st_i}")
        for kt in range(NTE):
            lhsT = oT[:, kt * S + st_i * P : kt * S + st_i * P + P]
            for j in range(NE):
                nc.tensor.matmul(
                    ps[:, j * FD : (j + 1) * FD],
                    lhsT,
                    wo_bf[:, kt * E + j * FD : kt * E + (j + 1) * FD],
                    start=(kt == 0),
                    stop=(kt == NTE - 1),
                )
        nc.vector.tensor_add(out=osb, in0=ps[:, :E], in1=bo_sb)
        nc.sync.dma_start(out=io["out"][st_i * P : (st_i + 1) * P, :], in_=osb)


def build_nc(S=1024, E=1024, H=16):
    key = (S, E, H)
    if key in _NC_CACHE:
        return _NC_CACHE[key]
    import concourse.tile as tile
    from concourse import bacc, mybir

    D = E // H
    HA = H * (D + 1)
    HPT = P // D
    f32 = mybir.dt.float32
    bf16 = mybir.dt.bfloat16
    nc = bacc.Bacc("TRN2", target_bir_lowering=False, debug=False)
    io = {}
    for name, shape, dt in [
        ("xqT", [E, S], bf16),
        ("xkT", [E, S], bf16),
        ("xvT", [E, S], bf16),
        ("wqT", [E, E], bf16),
        ("wkT", [E, E], bf16),
        ("wvTa", [E, HA], bf16),
        ("woT", [E, E], bf16),
        ("bq", [E], f32),
        ("bk", [E], f32),
        ("bva", [HA], f32),
        ("bo", [E], f32),
        ("sel", [HPT, P], f32),
    ]:
        io[name] = nc.dram_tensor(name, shape, dt, kind="ExternalInput").ap()
    io["out"] = nc.dram_tensor("out", [S, E], f32, kind="ExternalOutput").ap()

    with tile.TileContext(nc) as tc:
        with ExitStack() as ctx:
            _emit(ctx, tc, io, S, E, H)
    nc.compile()
    _NC_CACHE[key] = nc
    return nc


def make_in_maps(queries, keys, values, Wq, bq, Wk, bk, Wv, bv, Wo, bo, H=16):
    """Host-side layout prep: transposes, bf16 casts, v augmentation."""
    import ml_dtypes

    N, S, E = queries.shape
    D = E // H
    DA = D + 1
    HA = H * DA
    HPT = P // D
    f32 = np.float32
    bf16 = ml_dtypes.bfloat16

    wqT = np.ascontiguousarray(np.asarray(Wq, f32).T.astype(bf16))
    wkT = np.ascontiguousarray(np.asarray(Wk, f32).T.astype(bf16))
    woT = np.ascontiguousarray(np.asarray(Wo, f32).T.astype(bf16))
    wvT = np.asarray(Wv, f32).T.astype(bf16)  # [f, e]
    wvTa = np.zeros((E, HA), bf16)
    bva = np.zeros((HA,), f32)
    bv = np.asarray(bv, f32)
    for h in range(H):
        wvTa[:, h * DA : h * DA + D] = wvT[:, h * D : (h + 1) * D]
        bva[h * DA : h * DA + D] = bv[h * D : (h + 1) * D]
        bva[h * DA + D] = 1.0  # ones column -> softmax denominator
    sel = np.zeros((HPT, P), f32)
    for i in range(HPT):
        sel[i, i * D : (i + 1) * D] = 1.0
    shared = {
        "wqT": wqT,
        "wkT": wkT,
        "wvTa": wvTa,
        "woT": woT,
        "bq": np.ascontiguousarray(np.asarray(bq, f32)),
        "bk": np.ascontiguousarray(np.asarray(bk, f32)),
        "bva": bva,
        "bo": np.ascontiguousarray(np.asarray(bo, f32)),
        "sel": sel,
    }
    q = np.asarray(queries, f32)
    k = np.asarray(keys, f32)
    v = np.asarray(values, f32)
    in_maps = []
    for b in range(N):
        m = dict(shared)
        m["xqT"] = np.ascontiguousarray(q[b].T.astype(bf16))
        m["xkT"] = np.ascontiguousarray(k[b].T.astype(bf16))
        m["xvT"] = np.ascontiguousarray(v[b].T.astype(bf16))
        in_maps.append(m)
    return in_maps


def run(queries, keys, values, Wq, bq, Wk, bk, Wv, bv, Wo, bo, **spmd_kwargs):
    from concourse.bass_utils import run_bass_kernel_spmd

    queries = np.asarray(queries, np.float32)
    N, S, E = queries.shape
    H = 16
    nc = build_nc(S=S, E=E, H=H)
    in_maps = make_in_maps(queries, keys, values, Wq, bq, Wk, bk, Wv, bv, Wo, bo, H=H)
    res = run_bass_kernel_spmd(nc, in_maps, core_ids=list(range(N)), **spmd_kwargs)
    out = np.stack([res.results[b]["out"] for b in range(N)])
    return out.astype(np.float32), res


def kernel(queries, keys, values, Wq, bq, Wk, bk, Wv, bv, Wo, bo):
    out, _ = run(queries, keys, values, Wq, bq, Wk, bk, Wv, bv, Wo, bo)
    return out


# revision 27
# speedup vs baseline: 1.0402x; 1.0402x over previous
"""Trainium2 Bass kernel for nn_MultiHeadAttention (N=8, S=1024, E=1024, H=16).

Strategy: pure data-parallel over the batch dim N=8 -> one batch element per
NeuronCore, no collectives. Per core the whole MHA runs out of SBUF:

  v   = xv @ Wv_aug.T + bv_aug   (S-major [S, H*(D+1)] with a ones column
                                  appended per head -> o-matmul also yields
                                  the softmax denominator for free)
  q.T = Wq @ xq.T + bq      (E-major "transposed" layout [E, S])
  k.T = Wk @ xk.T + bk
  per head h (software-pipelined at s_k-tile granularity):
    scoresT[s_k, s_q] tile = k_h.T-slice x q_h.T   (PSUM)
    attnT = exp(scoresT/sqrt(E))  on ScalarE (the only exp engine; paces
                                   the attention phase at ~1us/tile)
    o_unnorm.T[d, s_q] (+ denom row) += v_aug_h^T @ attnT  (PSUM accum)
  normalization is deferred/batched: denominator rows collect in SBUF, a
  fast approx reciprocal runs per batch of heads, the reciprocal rows are
  broadcast across partitions with a tiny K=2 fp32 matmul (selector
  constant), and oT is scaled in place -- all off the PE critical path.
  out = o @ Wo.T + bo       (natural [S, E] layout, DMA to DRAM)

All big matmul operands are bf16 (fp32 accumulation in PSUM); inputs are
pre-transposed and pre-cast on the host (layout/precision prep only).
"""

import math
from contextlib import ExitStack

import numpy as np

P = 128  # SBUF partitions
FDMAX = 512  # matmul moving-operand free-dim tile

_NC_CACHE = {}


def _emit(ctx, tc, io, S, E, H):
    from concourse import mybir

    nc = tc.nc
    D = E // H
    DA = D + 1
    HA = H * DA
    NTE = E // P  # partition tiles over e/f dims
    NTS = S // P  # partition tiles over s dim
    FD = min(FDMAX, S)
    NQ = S // FD  # free tiles over s
    NE = E // FD  # free tiles over e
    HPT = P // D  # heads per e-tile
    f32 = mybir.dt.float32
    bf16 = mybir.dt.bfloat16

    singles = ctx.enter_context(tc.tile_pool(name="singles", bufs=1))
    xpool = ctx.enter_context(tc.tile_pool(name="xpool", bufs=2))
    wpool = ctx.enter_context(tc.tile_pool(name="wpool", bufs=2))
    atp = ctx.enter_context(tc.tile_pool(name="atp", bufs=4))
    outp = ctx.enter_context(tc.tile_pool(name="outp", bufs=2))
    mini = ctx.enter_context(tc.tile_pool(name="mini", bufs=2))
    psA = ctx.enter_context(tc.tile_pool(name="psA", bufs=4, space="PSUM"))

    # persistent bf16 activations; layout [row % P, tile_idx * width + col]
    qT = singles.tile([P, NTE * S], bf16)  # q.T [e, s]
    kT = singles.tile([P, NTE * S], bf16)  # k.T [e, s]
    vA = singles.tile([P, NTS * HA], bf16)  # v_aug [s, HA]
    oT = singles.tile([P, NTE * S], bf16)  # o.T [e, s]

    # ---- input loads, in consumption order (DMA queues drain in order) ----
    xv_bf = xpool.tile([P, NTE * S], bf16, tag="x")
    wv_bf = wpool.tile([P, NTE * max(E, HA)], bf16, tag="w")
    for t in range(NTE):
        nc.sync.dma_start(
            out=wv_bf[:, t * HA : (t + 1) * HA], in_=io["wvTa"][t * P : (t + 1) * P, :]
        )
        nc.gpsimd.dma_start(
            out=xv_bf[:, t * S : (t + 1) * S], in_=io["xvT"][t * P : (t + 1) * P, :]
        )
    xq_bf = xpool.tile([P, NTE * S], bf16, tag="x")
    wq_bf = wpool.tile([P, NTE * max(E, HA)], bf16, tag="w")
    for t in range(NTE):
        nc.sync.dma_start(
            out=wq_bf[:, t * E : (t + 1) * E], in_=io["wqT"][t * P : (t + 1) * P, :]
        )
        nc.gpsimd.dma_start(
            out=xq_bf[:, t * S : (t + 1) * S], in_=io["xqT"][t * P : (t + 1) * P, :]
        )
    xk_bf = xpool.tile([P, NTE * S], bf16, tag="x")
    wk_bf = wpool.tile([P, NTE * max(E, HA)], bf16, tag="w")
    for t in range(NTE):
        nc.sync.dma_start(
            out=wk_bf[:, t * E : (t + 1) * E], in_=io["wkT"][t * P : (t + 1) * P, :]
        )
        nc.gpsimd.dma_start(
            out=xk_bf[:, t * S : (t + 1) * S], in_=io["xkT"][t * P : (t + 1) * P, :]
        )
    wo_bf = wpool.tile([P, NTE * max(E, HA)], bf16, tag="w")
    for t in range(NTE):
        eng = nc.sync if t % 2 == 0 else nc.gpsimd
        eng.dma_start(
            out=wo_bf[:, t * E : (t + 1) * E], in_=io["woT"][t * P : (t + 1) * P, :]
        )

    # biases + selector constant (scalar queue; small)
    bq_sb = singles.tile([P, NTE], f32)
    bk_sb = singles.tile([P, NTE], f32)
    bq2 = io["bq"].rearrange("(a b) -> a b", b=1)
    bk2 = io["bk"].rearrange("(a b) -> a b", b=1)
    for et in range(NTE):
        nc.scalar.dma_start(
            out=bq_sb[:, et : et + 1], in_=bq2[et * P : (et + 1) * P, :]
        )
        nc.scalar.dma_start(
            out=bk_sb[:, et : et + 1], in_=bk2[et * P : (et + 1) * P, :]
        )
    bva_sb = singles.tile([P, HA], f32)
    nc.scalar.dma_start(
        out=bva_sb, in_=io["bva"].rearrange("(a b) -> a b", a=1).to_broadcast((P, HA))
    )
    bo_sb = singles.tile([P, E], f32)
    nc.scalar.dma_start(
        out=bo_sb, in_=io["bo"].rearrange("(a b) -> a b", a=1).to_broadcast((P, E))
    )
    sel_sb = singles.tile([HPT, P], f32)
    nc.scalar.dma_start(out=sel_sb, in_=io["sel"])

    # ---- v projection: v_aug[s, c] = sum_f xv[f, s] * wv_aug[f, c] + bva ----
    main_w = (HA // FD) * FD
    tail_w = HA - main_w
    for st_i in range(NTS):
        ps_main = psA.tile([P, max(S, main_w)], f32, tag="psA")
        ps_tail = (
            psA.tile([P, max(S, main_w)], f32, tag="psA", name=f"ps_tail_{st_i}")
            if tail_w
            else None
        )
        for kt in range(NTE):
            lhsT = xv_bf[:, kt * S + st_i * P : kt * S + st_i * P + P]
            for j in range(main_w // FD):
                nc.tensor.matmul(
                    ps_main[:, j * FD : (j + 1) * FD],
                    lhsT,
                    wv_bf[:, kt * HA + j * FD : kt * HA + (j + 1) * FD],
                    start=(kt == 0),
                    stop=(kt == NTE - 1),
                )
            if ps_tail is not None:
                nc.tensor.matmul(
                    ps_tail[:, :tail_w],
                    lhsT,
                    wv_bf[:, kt * HA + main_w : kt * HA + HA],
                    start=(kt == 0),
                    stop=(kt == NTE - 1),
                )
        nc.vector.tensor_add(
            out=vA[:, st_i * HA : st_i * HA + main_w],
            in0=ps_main[:, :main_w],
            in1=bva_sb[:, :main_w],
        )
        if ps_tail is not None:
            nc.vector.tensor_add(
                out=vA[:, st_i * HA + main_w : (st_i + 1) * HA],
                in0=ps_tail[:, :tail_w],
                in1=bva_sb[:, main_w:HA],
            )

    # ---- q/k projections: dst[e, s] = sum_f w[f, e] x[f, s] + b[e] ----
    def project_qk(dst, w_bf, x_bf, bias_sb):
        for et in range(NTE):
            ps = psA.tile([P, max(S, main_w)], f32, tag="psA")
            for kt in range(NTE):
                lhsT = w_bf[:, kt * E + et * P : kt * E + (et + 1) * P]
                for j in range(NQ):
                    nc.tensor.matmul(
                        ps[:, j * FD : (j + 1) * FD],
                        lhsT,
                        x_bf[:, kt * S + j * FD : kt * S + (j + 1) * FD],
                        start=(kt == 0),
                        stop=(kt == NTE - 1),
                    )
            nc.vector.tensor_scalar_add(
                out=dst[:, et * S : (et + 1) * S],
                in0=ps[:, :S],
                scalar1=bias_sb[:, et : et + 1],
            )

    project_qk(qT, wq_bf, xq_bf, bq_sb)
    project_qk(kT, wk_bf, xk_bf, bk_sb)

    # ---- attention, kt-granular software pipeline per head ----
    inv_scale = 1.0 / math.sqrt(E)
    nbatch = 4 if H % 4 == 0 else 1
    hb = H // nbatch
    den_batches = [
        singles.tile([hb, S], f32, name=f"den_batch{b}") for b in range(nbatch)
    ]

    def normalize_et(et):
        # rb[p, s] = 1/den[head(p), s] replicated via K=HPT fp32 matmul
        den2 = mini.tile([HPT, S], f32, tag="den2", name=f"den2_{et}")
        for i in range(HPT):
            hh = et * HPT + i
            nc.gpsimd.dma_start(
                out=den2[i : i + 1, :],
                in_=den_batches[hh // hb][hh % hb : hh % hb + 1, :],
            )
        rb = psA.tile([P, max(S, main_w)], f32, tag="psA", name=f"rb_{et}")
        for j in range(NQ):
            nc.tensor.matmul(
                rb[:, j * FD : (j + 1) * FD],
                sel_sb,
                den2[:, j * FD : (j + 1) * FD],
                start=True,
                stop=True,
            )
        nc.vector.tensor_mul(
            out=oT[:, et * S : (et + 1) * S],
            in0=oT[:, et * S : (et + 1) * S],
            in1=rb[:, :S],
        )

    for h in range(H):
        eh = (h * D) // P
        ph = (h * D) % P
        pso = psA.tile([P, max(S, main_w)], f32, tag="psA", name=f"pso_{h}")
        ats = []
        # scores/exp for kt, with o-matmuls trailing one kt behind
        for kt in range(NTS):
            ps = psA.tile([P, max(S, main_w)], f32, tag="psA", name=f"sc_{h}_{kt}")
            lhsT = kT[ph : ph + D, eh * S + kt * P : eh * S + (kt + 1) * P]
            for j in range(NQ):
                nc.tensor.matmul(
                    ps[:, j * FD : (j + 1) * FD],
                    lhsT,
                    qT[ph : ph + D, eh * S + j * FD : eh * S + (j + 1) * FD],
                    start=True,
                    stop=True,
                )
            at = atp.tile([P, S], bf16, tag="at", name=f"at_{h}_{kt}")
            nc.scalar.activation(
                out=at,
                in_=ps[:, :S],
                func=mybir.ActivationFunctionType.Exp,
                scale=inv_scale,
            )
            ats.append(at)

            def o_mms(okt):
                lhsTo = vA[:, okt * HA + h * DA : okt * HA + (h + 1) * DA]
                for j in range(NQ):
                    nc.tensor.matmul(
                        pso[:DA, j * FD : (j + 1) * FD],
                        lhsTo,
                        ats[okt][:, j * FD : (j + 1) * FD],
                        start=(okt == 0),
                        stop=(okt == NTS - 1),
                    )

            if kt >= 1:
                o_mms(kt - 1)
        o_mms(NTS - 1)
        # evacuate unnormalized o (bf16) + denominator row
        nc.vector.tensor_copy(
            out=oT[ph : ph + D, eh * S : (eh + 1) * S], in_=pso[:D, :S]
        )
        den_tmp = mini.tile([1, S], f32, tag="den_tmp")
        nc.vector.tensor_copy(out=den_tmp, in_=pso[D:DA, :S])
        nc.gpsimd.dma_start(
            out=den_batches[h // hb][h % hb : h % hb + 1, :], in_=den_tmp
        )
        if h % hb == hb - 1:
            b = h // hb
            nc.vector.reciprocal_approx_fast(
                out=den_batches[b], in_=den_batches[b]
            )
            # normalize with ONE BATCH of delay: batch b-1's reciprocal chain
            # (DVE+DMA) has had a full batch of PE work to complete, so the
            # rb matmuls never stall the in-order PE queue.
            if b > 0:
                for et in range(((b - 1) * hb) // HPT, (b * hb) // HPT):
                    normalize_et(et)
            if h == H - 1:
                for et in range((b * hb) // HPT, (h + 1) // HPT):
                    normalize_et(et)

    # ---- output projection: out[s, e] = sum_f oT[f, s] woT[f, e] + bo ----
    for st_i in range(NTS):
        osb = outp.tile([P, E], f32, tag="out", name=f"osb_{st_i}")
        ps = psA.tile([P, max(S, main_w)], f32, tag="psA", name=f"po_{st_i}")
        for kt in range(NTE):
            lhsT = oT[:, kt * S + st_i * P : kt * S + st_i * P + P]
            for j in range(NE):
                nc.tensor.matmul(
                    ps[:, j * FD : (j + 1) * FD],
                    lhsT,
                    wo_bf[:, kt * E + j * FD : kt * E + (j + 1) * FD],
                    start=(kt == 0),
                    stop=(kt == NTE - 1),
                )
        nc.vector.tensor_add(out=osb, in0=ps[:, :E], in1=bo_sb)
        nc.sync.dma_start(out=io["out"][st_i * P : (st_i + 1) * P, :], in_=osb)


def build_nc(S=1024, E=1024, H=16):
    key = (S, E, H)
    if key in _NC_CACHE:
        return _NC_CACHE[key]
    import concourse.tile as tile
    from concourse import bacc, mybir

    D = E // H
    HA = H * (D + 1)
    HPT = P // D
    f32 = mybir.dt.float32
    bf16 = mybir.dt.bfloat16
    nc = bacc.Bacc("TRN2", target_bir_lowering=False, debug=False)
    io = {}
    for name, shape, dt in [
        ("xqT", [E, S], bf16),
        ("xkT", [E, S], bf16),
        ("xvT", [E, S], bf16),
        ("wqT", [E, E], bf16),
        ("wkT", [E, E], bf16),
        ("wvTa", [E, HA], bf16),
        ("woT", [E, E], bf16),
        ("bq", [E], f32),
        ("bk", [E], f32),
        ("bva", [HA], f32),
        ("bo", [E], f32),
        ("sel", [HPT, P], f32),
    ]:
        io[name] = nc.dram_tensor(name, shape, dt, kind="ExternalInput").ap()
    io["out"] = nc.dram_tensor("out", [S, E], f32, kind="ExternalOutput").ap()

    with tile.TileContext(nc) as tc:
        with ExitStack() as ctx:
            _emit(ctx, tc, io, S, E, H)
    nc.compile()
    _NC_CACHE[key] = nc
    return nc


def make_in_maps(queries, keys, values, Wq, bq, Wk, bk, Wv, bv, Wo, bo, H=16):
    """Host-side layout prep: transposes, bf16 casts, v augmentation."""
    import ml_dtypes

    N, S, E = queries.shape
    D = E // H
    DA = D + 1
    HA = H * DA
    HPT = P // D
    f32 = np.float32
    bf16 = ml_dtypes.bfloat16

    wqT = np.ascontiguousarray(np.asarray(Wq, f32).T.astype(bf16))
    wkT = np.ascontiguousarray(np.asarray(Wk, f32).T.astype(bf16))
    woT = np.ascontiguousarray(np.asarray(Wo, f32).T.astype(bf16))
    wvT = np.asarray(Wv, f32).T.astype(bf16)  # [f, e]
    wvTa = np.zeros((E, HA), bf16)
    bva = np.zeros((HA,), f32)
    bv = np.asarray(bv, f32)
    for h in range(H):
        wvTa[:, h * DA : h * DA + D] = wvT[:, h * D : (h + 1) * D]
        bva[h * DA : h * DA + D] = bv[h * D : (h + 1) * D]
        bva[h * DA + D] = 1.0  # ones column -> softmax denominator
    sel = np.zeros((HPT, P), f32)
    for i in range(HPT):
        sel[i, i * D : (i + 1) * D] = 1.0
    shared = {
        "wqT": wqT,
        "wkT": wkT,
        "wvTa": wvTa,
        "woT": woT,
        "bq": np.ascontiguousarray(np.asarray(bq, f32)),
        "bk": np.ascontiguousarray(np.asarray(bk, f32)),
        "bva": bva,
        "bo": np.ascontiguousarray(np.asarray(bo, f32)),
        "sel": sel,
    }
    q = np.asarray(queries, f32)
    k = np.asarray(keys, f32)
    v = np.asarray(values, f32)
    in_maps = []
    for b in range(N):
        m = dict(shared)
        m["xqT"] = np.ascontiguousarray(q[b].T.astype(bf16))
        m["xkT"] = np.ascontiguousarray(k[b].T.astype(bf16))
        m["xvT"] = np.ascontiguousarray(v[b].T.astype(bf16))
        in_maps.append(m)
    return in_maps


def run(queries, keys, values, Wq, bq, Wk, bk, Wv, bv, Wo, bo, **spmd_kwargs):
    from concourse.bass_utils import run_bass_kernel_spmd

    queries = np.asarray(queries, np.float32)
    N, S, E = queries.shape
    H = 16
    nc = build_nc(S=S, E=E, H=H)
    in_maps = make_in_maps(queries, keys, values, Wq, bq, Wk, bk, Wv, bv, Wo, bo, H=H)
    res = run_bass_kernel_spmd(nc, in_maps, core_ids=list(range(N)), **spmd_kwargs)
    out = np.stack([res.results[b]["out"] for b in range(N)])
    return out.astype(np.float32), res


def kernel(queries, keys, values, Wq, bq, Wk, bk, Wv, bv, Wo, bo):
    out, _ = run(queries, keys, values, Wq, bq, Wk, bk, Wv, bv, Wo, bo)
    return out
